# revision 8
# baseline (speedup 1.0000x reference)
"""Trainium2 Bass kernel for nn_MIPS_74904229642848 (v3).

Pipeline (8 NeuronCores, SPMD, batch-sharded 2 rows/core, S=4 streams/core):
  1. 2-layer bidirectional LSTM, all-bf16 matmuls. Per step: one identity
     matmul injects the precomputed input-gate terms into PSUM (chunked so
     each recurrence starts after its first xg chunk), four bf16 block-diag
     recurrence matmuls accumulate on top. The g-gate weights/bias are
     pre-scaled by 2 on the host so ONE sigmoid over all four gates covers
     the tanh as well (tanh(g) = 2*sigmoid(2g)-1); the cell update is two
     fused scalar_tensor_tensor ops on DVE (m1' = (u-.5)*si;
     c = 2*m1' + m2) with m2 = sf*c on GpSimd, c in SBUF, tanh(c) on ACT,
     h = so*tanh(c) on DVE into a triple-buffered Hst.
  2. L2 normalization via ln/exp rsqrt (no Newton), fused scale+pack.
  3. Windowed index sampling of z2 via indirect DMA (bf16).
  4. AllGather of the B-side embeddings only (bf16).
  5. One-pass logits sweep: bf16 sim matmul blocks, exp (ACT, accum_out
     gives row sums), ones-matmul accumulates column sums in PSUM across
     row blocks. Row-lse finished on device; per-core column-sum partials
     shipped to the host, which does the final ln+sum combine.
"""

import numpy as np

_D, _E, _H, _B, _W = 64, 128, 64, 16, 3
_T = 512
_TEMP = 0.05
_NCORES = 8
_BS = _B // _NCORES          # batch rows per core
_S = 2 * _BS                 # streams per core: (x1,b0),(x1,b1),(x2,b0),(x2,b1)
_GF = 4 * _S                 # gate-block width per step

# torch gate order i,f,g,o -> kernel order o,i,f,g (tanh block last)
_GPERM = [3, 0, 1, 2]
_GTANH = 3                   # index of the g gate in kernel order

_cache = {}


def _build(T, dbg=False):
    import concourse.bass as bass
    import concourse.mybir as mybir
    import concourse.tile as tile
    from concourse import bacc, library_config
    from concourse.masks import make_identity

    f32 = mybir.dt.float32
    bf16 = mybir.dt.bfloat16
    i32 = mybir.dt.int32
    AF = mybir.ActivationFunctionType
    OP = mybir.AluOpType

    S = _S
    GF = _GF
    TS = T * S
    NLOC = _BS * T
    NGLOB = _NCORES * NLOC
    FC = min(512, TS)            # xg free chunk
    NFC = TS // FC
    TCH = FC // S                # timesteps per xg chunk
    CC = min(512, NGLOB)         # logits col chunk
    NCC = NGLOB // CC
    NRC = (NLOC + 127) // 128    # logits row chunks (M=128)
    NTC = T // 128               # transpose chunks per stream
    NUC = T // 128

    nc = bacc.Bacc("TRN2", target_bir_lowering=False, debug=False,
                   num_devices=_NCORES)

    # ---- I/O ----
    x_in = nc.dram_tensor("x_cat", [_D, TS], bf16, kind="ExternalInput")
    xr_in = nc.dram_tensor("x_rev", [_D, TS], bf16, kind="ExternalInput")
    u2_in = nc.dram_tensor("u2", [1, _BS * T], f32, kind="ExternalInput")
    tw_in = nc.dram_tensor("tw2", [1, _BS * T], f32, kind="ExternalInput")
    wi0_in = nc.dram_tensor("wi0T", [_D, 512], bf16, kind="ExternalInput")
    wi1_in = nc.dram_tensor("wi1T", [_E, 512], bf16, kind="ExternalInput")
    wh0_in = nc.dram_tensor("wh0bd", [_E, 512], bf16, kind="ExternalInput")
    wh1_in = nc.dram_tensor("wh1bd", [_E, 512], bf16, kind="ExternalInput")
    b0_in = nc.dram_tensor("bias0", [_E, 4], f32, kind="ExternalInput")
    b1_in = nc.dram_tensor("bias1", [_E, 4], f32, kind="ExternalInput")
    out_d = nc.dram_tensor("outp", [128, 4], f32, kind="ExternalOutput")
    cs_d = nc.dram_tensor("colsum", [1, NGLOB], f32, kind="ExternalOutput")
    if dbg:
        anb_d = nc.dram_tensor("anb", [128, 4 * NLOC], f32,
                               kind="ExternalOutput")

    # DRAM scratch for the AllGather
    ag_in_d = nc.dram_tensor("ag_in", [128, NLOC], bf16)
    ag_out_d = nc.dram_tensor("ag_out", [_NCORES * 128, NLOC], bf16,
                              addr_space="Shared")

    with tile.TileContext(nc) as tc:
        with (
            tc.tile_pool(name="consts", bufs=1) as consts,
            tc.tile_pool(name="bigbuf", bufs=1) as bigbuf,
            tc.tile_pool(name="state", bufs=1) as state,
            tc.tile_pool(name="small", bufs=3) as small,
            tc.tile_pool(name="nrm", bufs=1) as nrm,
            tc.tile_pool(name="sg", bufs=4) as sgp,
        ):
            dma = nc.sync.dma_start

            # ---------- load constants / inputs ----------
            x_sb = bigbuf.tile([_D, TS], bf16, tag="x")
            xr_sb = bigbuf.tile([_D, TS], bf16, tag="xr")
            dma(out=x_sb[:], in_=x_in.ap())
            dma(out=xr_sb[:], in_=xr_in.ap())
            wi0_sb = consts.tile([_D, 512], bf16)
            wi1_sb = consts.tile([_E, 512], bf16)
            wh0_sb = consts.tile([_E, 512], bf16)
            wh1_sb = consts.tile([_E, 512], bf16)
            b0_sb = consts.tile([_E, 4], f32)
            b1_sb = consts.tile([_E, 4], f32)
            for sb, di in ((wi0_sb, wi0_in), (wi1_sb, wi1_in),
                           (wh0_sb, wh0_in), (wh1_sb, wh1_in),
                           (b0_sb, b0_in), (b1_sb, b1_in)):
                dma(out=sb[:], in_=di.ap())
            ident = consts.tile([128, 128], bf16)
            make_identity(nc, ident[:])
            # dummy sigmoid as the first ACT op: loads the sigmoid table
            # set (which also serves Identity) during the input DMAs, so
            # neither the xg writes nor recurrence step 0 pay a table load
            warm_act = consts.tile([1, 1], f32)
            nc.vector.memset(warm_act[:], 0.0)
            nc.scalar.activation(warm_act[:], warm_act[:], AF.Sigmoid)
            ones_col = consts.tile([128, 1], bf16)
            nc.vector.memset(ones_col[:], 1.0)
            ones_row = consts.tile([1, 128], f32)
            nc.vector.memset(ones_row[:], 1.0)
            ones_rowb = consts.tile([1, 128], bf16)
            nc.vector.memset(ones_rowb[:], 1.0)

            # ---------- window offsets on device (only needs u) ----------
            # o[b,t] = i2[b,t] - t in [-3, 4]; the z2 sampling is then 8
            # one-hot masked shift-accumulates (no gpsimd gather needed)
            # rows b live at partitions 0 and 32 (legal matmul rhs bases);
            # the ops run on the full [64, T] view so they stream 512-wide
            ob = consts.tile([64, T], bf16)
            with tc.tile_pool(name="idxp", bufs=1) as idxp:
                u2 = idxp.tile([64, T], f32, name="u2t")
                tf = idxp.tile([64, T], f32, name="tft")
                for b in range(_BS):
                    dma(out=u2[32 * b:32 * b + 1, :],
                        in_=u2_in.ap()[0:1, b * T:(b + 1) * T])
                    dma(out=tf[32 * b:32 * b + 1, :],
                        in_=tw_in.ap()[0:1, b * T:(b + 1) * T])
                ks = idxp.tile([64, T], f32, name="kst")
                nc.vector.tensor_scalar_mul(ks[:], tf[:], 1.0 / (T - 1))
                # the oracle's (k*s).astype(int32) rounds-to-nearest on
                # neuron, so the carry fires at 0.5 rather than 1.0
                nc.vector.tensor_scalar(ks[:], ks[:], 0.5, None, OP.is_ge)
                nc.vector.tensor_add(ks[:], tf[:], ks[:])      # center
                lo = idxp.tile([64, T], f32, name="lot")
                nc.vector.tensor_scalar(lo[:], ks[:], -float(_W), 0.0,
                                        OP.add, OP.max)
                hi = idxp.tile([64, T], f32, name="hit")
                nc.vector.tensor_scalar(hi[:], ks[:], float(_W), float(T),
                                        OP.add, OP.min)
                cnt = idxp.tile([64, T], f32, name="cntt")
                nc.vector.tensor_sub(cnt[:], hi[:], lo[:])
                nc.vector.tensor_mul(cnt[:], u2[:], cnt[:])    # pr
                fr = idxp.tile([64, T], f32, name="frt")
                nc.vector.memset(fr[:], 0.0)
                for kth in range(1, 2 * _W + 1):
                    nc.vector.scalar_tensor_tensor(
                        fr[:], cnt[:], float(kth), fr[:], OP.is_ge, OP.add)
                nc.vector.tensor_add(lo[:], lo[:], fr[:])      # i2 (pre-min)
                nc.vector.tensor_scalar_add(hi[:], hi[:], -1.0)
                nc.vector.tensor_tensor(lo[:], lo[:], hi[:], op=OP.min)
                nc.vector.tensor_sub(lo[:], lo[:], tf[:])      # o = i2 - t
                nc.vector.tensor_copy(ob[:], lo[:])
            # broadcast o across partitions (1-contraction matmuls) and
            # build the 8 one-hot masks; overlaps the LSTM epilogue
            ones_rowb0 = consts.tile([64, 128], bf16)
            nc.vector.memset(ones_rowb0[:], 1.0)
            # mask storage is aliased into the XG tiles and H0r, which
            # are dead once the layer-2 recurrence has consumed them

            # ---------- LSTM ----------
            def xg_precompute(ps_big, wiT_sb, K, srcs_f, srcs_b, bias_sb,
                              XGs):
                # XG layout: [128, (t, gate, s)] with gf = GF per step,
                # chunked into NFC tiles so the recurrence can start as
                # soon as chunk 0 is written
                for ch in range(NFC):
                    XG3 = XGs[ch][:].rearrange("p (t gf) -> p t gf", gf=GF)
                    for g in range(4):
                        ps = ps_big.tile([128, FC], f32, tag="xgps")
                        nc.tensor.matmul(
                            out=ps[0:_H, :],
                            lhsT=wiT_sb[0:K, g * 64:(g + 1) * 64],
                            rhs=srcs_f[ch],
                            start=True, stop=True)
                        nc.tensor.matmul(
                            out=ps[_H:128, :],
                            lhsT=wiT_sb[0:K, 256 + g * 64:256 + (g + 1) * 64],
                            rhs=srcs_b[ch],
                            start=True, stop=True)
                        dst = XG3[:, :, g * S:(g + 1) * S]
                        nc.scalar.activation(
                            dst,
                            ps[:].rearrange("p (t s) -> p t s", s=S),
                            AF.Identity, bias=bias_sb[:, g:g + 1])

            def recurrence(ps_gate, wh_sb, XGs, Ht, Hr=None, lyr=0):
                # Cell state via tensor_tensor_scan: we track ct = c/2, so
                #   ct_t = sf * ct_{t-1} + m1',  m1' = (u-0.5)*si
                #        (= [sf*c + si*tanh(g)] / 2 since tanh(g) = 2u-1)
                #   tanh(c) = Tanh(2*ct)  (ACT scale=2)
                # The scan's free layout interleaves a reset slot (d0=0,
                # d1=ct_prev) and an update slot (d0=sf, d1=m1') per stream.
                # sigma writes all gates strided into sgx (odd cols); even
                # cols stay 0 so sgx[8:16] is [0 sf 0 sf ...] = the scan d0.
                # R tiles: scan(t) writes [e0 n0 e1 n1 ...] into R[t%2][0:8]
                # (n_s = new ct at col 2s+1); m1'(t+1) then lands at cols
                # {2,4,6,8} of the same tile, so R[t%2][1:9] is the next d1.
                Hsts = [state.tile([128, S], bf16, tag=f"hst{lyr}{i}",
                                   name=f"hst{lyr}{i}") for i in range(3)]
                Rs = [state.tile([128, 2 * S + 2], f32, tag=f"r{lyr}{i}",
                                 name=f"r{lyr}{i}") for i in range(2)]
                Tc = [state.tile([128, 2 * S], bf16, tag=f"tc{lyr}{i}",
                                 name=f"tc{lyr}{i}") for i in range(2)]
                Sgx = [state.tile([128, 2 * GF], f32, tag=f"sg{lyr}{i}",
                                  name=f"sg{lyr}{i}") for i in range(3)]
                for i in range(3):
                    nc.vector.memset(Hsts[i][:], 0.0)
                for i in range(2):
                    nc.vector.memset(Rs[i][:], 0.0)
                for i in range(3):
                    nc.vector.memset(Sgx[i][:], 0.0)

                def sview(sgx, g):
                    # [128, S, 1] view of gate g's columns {2*(g*S+s)+1}
                    v = sgx[:].rearrange("p (c z) -> p c z", z=2)
                    return v[:, g * S:(g + 1) * S, 1:2]

                def emit_copies(t, Hst):
                    # Ht copies on DVE right after h (in-order, no sem);
                    # Hr copies on Pool read Ht, so the h-write never
                    # carries a Pool anti-dependency wait
                    rt = T - 1 - t
                    nc.vector.tensor_copy(Ht[0:_H, t * S:(t + 1) * S],
                                          Hst[0:_H, :])
                    nc.vector.tensor_copy(Ht[_H:128, rt * S:(rt + 1) * S],
                                          Hst[_H:128, :])
                    if Hr is not None:
                        nc.gpsimd.tensor_copy(
                            Hr[0:_H, rt * S:(rt + 1) * S],
                            Ht[0:_H, t * S:(t + 1) * S])
                        nc.gpsimd.tensor_copy(
                            Hr[_H:128, t * S:(t + 1) * S],
                            Ht[_H:128, rt * S:(rt + 1) * S])

                for t in range(T):
                    Hprev = Hsts[(t + 2) % 3]
                    Hst = Hsts[t % 3]
                    Ra = Rs[t % 2]
                    Rb = Rs[(t + 1) % 2]
                    tc = Tc[t % 2]
                    sgx = Sgx[t % 3]
                    gb = ps_gate.tile([128, GF], f32, tag="gates")
                    xgsl = XGs[t // TCH][:, (t % TCH) * GF:
                                         (t % TCH + 1) * GF]
                    # inject xg via identity matmul (clears has_written)
                    nc.tensor.matmul(out=gb[:], lhsT=ident[:],
                                     rhs=xgsl,
                                     start=True, stop=False,
                                     skip_group_check=True)
                    for g in (1, 2, 3, 0):
                        nc.tensor.matmul(
                            out=gb[:, g * S:(g + 1) * S],
                            lhsT=wh_sb[:, g * 128:(g + 1) * 128],
                            rhs=Hprev[:],
                            start=False, stop=(g == 0),
                            skip_group_check=True)
                    # sigmoid covers all gates (g-gate weights x2 on host,
                    # so its col holds u = sigmoid(2g)); split so the
                    # (i,f,g) part fires before the o-gate matmul lands;
                    # output is strided into odd cols of sgx
                    gb3 = gb[:, S:GF].rearrange("p (c z) -> p c z", z=1)
                    sgv = sgx[:].rearrange("p (c z) -> p c z", z=2)
                    nc.scalar.activation(sgv[:, S:GF, 1:2], gb3, AF.Sigmoid)
                    gb3o = gb[:, 0:S].rearrange("p (c z) -> p c z", z=1)
                    nc.scalar.activation(sgv[:, 0:S, 1:2], gb3o, AF.Sigmoid)
                    # m1' = (u - 0.5) * si into Rb cols {2,4,6,8}
                    m1o = Rb[:, 2:2 * S + 2].rearrange(
                        "p (s z) -> p s z", z=2)[:, :, 0:1]
                    nc.vector.scalar_tensor_tensor(
                        m1o, sview(sgx, 3), 0.5, sview(sgx, 1),
                        OP.subtract, OP.mult)
                    # ct scan: d0 = [0 sf 0 sf ...], d1 = [ct_prev m1' ...]
                    nc.vector.tensor_tensor_scan(
                        Ra[:, 0:2 * S], sgx[:, 4 * S:6 * S],
                        Rb[:, 1:2 * S + 1], 0.0, OP.mult, OP.add)
                    # tanh(c) = Tanh(2*ct) over the contiguous Ra window
                    # (odd slots are m1' junk, harmlessly tanh'd); the real
                    # values land at even cols of tc
                    nc.scalar.activation(tc[:], Ra[:, 1:2 * S + 1],
                                         AF.Tanh, scale=2.0)
                    tcv = tc[:].rearrange("p (s z) -> p s z", z=2)[:, :, 0:1]
                    hst3 = Hst[:].rearrange("p (s z) -> p s z", z=1)
                    nc.vector.tensor_tensor(hst3, sview(sgx, 0), tcv,
                                            op=OP.mult)
                    emit_copies(t, Hst)

            H0t = bigbuf.tile([128, TS], bf16, tag="h0t")
            H1t = bigbuf.tile([128, TS], bf16, tag="h1t")
            with (
                tc.tile_pool(name="ps_big", bufs=2, space="PSUM") as ps_big,
                tc.tile_pool(name="ps_gate", bufs=3, space="PSUM") as ps_gate,
            ):
                XGs = [bigbuf.tile([128, TCH * GF], bf16, tag=f"xg{ch}",
                                   name=f"xg{ch}")
                       for ch in range(NFC)]
                H0r = bigbuf.tile([128, TS], bf16, tag="h0r")
                srcs_f = [x_sb[:, ch * FC:(ch + 1) * FC] for ch in range(NFC)]
                srcs_b = [xr_sb[:, ch * FC:(ch + 1) * FC] for ch in range(NFC)]
                xg_precompute(ps_big, wi0_sb, _D, srcs_f, srcs_b, b0_sb, XGs)
                recurrence(ps_gate, wh0_sb, XGs, H0t, H0r, lyr=0)

                # layer 2: fwd reads H0t, bwd reads the reversed copy H0r
                # (negative-stride APs are rejected by the BIR verifier)
                srcs_f = [H0t[:, ch * FC:(ch + 1) * FC] for ch in range(NFC)]
                srcs_b = [H0r[:, ch * FC:(ch + 1) * FC] for ch in range(NFC)]
                xg_precompute(ps_big, wi1_sb, _E, srcs_f, srcs_b, b1_sb, XGs)
                recurrence(ps_gate, wh1_sb, XGs, H1t, lyr=1)

            # ---------- normalize + pack An (z1) / Z2 ----------
            An = bigbuf.tile([128, NLOC], bf16, tag="an")
            Z2f = bigbuf.tile([128, NLOC + 8], bf16, tag="z2")
            nc.vector.memset(Z2f[:, 0:4], 0.0)
            nc.vector.memset(Z2f[:, NLOC + 4:NLOC + 8], 0.0)
            Bn = bigbuf.tile([128, NLOC], bf16, tag="bn")
            masks = [XGs[dd // 2][:, (dd % 2) * NLOC:(dd % 2 + 1) * NLOC]
                     for dd in range(2 * _W + 2)]
            osb = H0r[:, 0:NLOC]
            tmpb = H0r[:, NLOC:2 * NLOC]
            H13 = H1t[:].rearrange("p (t s) -> p t s", s=S)
            n2 = nrm.tile([1, TS], f32, tag="n2")
            sq = bigbuf.tile([128, FC], bf16, tag="sq")
            with (
                tc.tile_pool(name="ps_nrm", bufs=4, space="PSUM") as ps_nrm,
                tc.tile_pool(name="ps_row", bufs=2, space="PSUM") as ps_row,
            ):
                for ch in range(NFC):
                    nc.vector.tensor_mul(sq[:],
                                         H1t[:, ch * FC:(ch + 1) * FC],
                                         H1t[:, ch * FC:(ch + 1) * FC])
                    psn = ps_row.tile([1, FC], f32, tag="psn")
                    nc.tensor.matmul(out=psn[:], lhsT=ones_col[:],
                                     rhs=sq[:], start=True, stop=True)
                    nc.vector.tensor_copy(n2[:, ch * FC:(ch + 1) * FC],
                                          psn[:])
                nc.vector.tensor_scalar_max(n2[:], n2[:], 1e-24)
                lnb = nrm.tile([1, TS], f32, tag="lnb")
                nc.scalar.activation(lnb[:], n2[:], AF.Ln)
                rin = nrm.tile([1, TS], bf16, tag="rin")
                nc.scalar.activation(rin[:], lnb[:], AF.Exp, scale=-0.5)
                # one-hot masks for the sampling shifts: built after the
                # norm reduction so rin is never queued behind them
                with tc.tile_pool(name="ps_ob", bufs=1,
                                  space="PSUM") as ps_ob:
                    for b in range(_BS):
                        pso = ps_ob.tile([128, T], f32, tag="pso")
                        nc.tensor.matmul(
                            out=pso[:],
                            lhsT=ones_rowb0[32 * b:32 * b + 1, :],
                            rhs=ob[32 * b:32 * b + 1, :],
                            start=True, stop=True)
                        nc.vector.tensor_copy(osb[:, b * T:(b + 1) * T],
                                              pso[:])
                for dd in range(2 * _W + 2):
                    nc.vector.tensor_scalar(masks[dd], osb[:],
                                            float(dd - _W), None,
                                            OP.is_equal)
                # pack Z2 first: the gather + chunked AllGather start as
                # early as possible; An is packed during the collectives
                def pack(streams, ch, psb):
                    ps3 = psb[:].rearrange("p (t s) -> p t s", s=S)
                    h3 = H13[:, ch * TCH:(ch + 1) * TCH, :]
                    for s in streams:
                        if s < _BS:
                            dst = An[:, s * T + ch * TCH:
                                     s * T + (ch + 1) * TCH]
                        else:
                            b = s - _BS
                            dst = Z2f[:, 4 + b * T + ch * TCH:
                                      4 + b * T + (ch + 1) * TCH]
                        nc.vector.tensor_tensor(
                            dst,
                            h3[:, :, s:s + 1].rearrange("p t o -> p (t o)"),
                            ps3[:, :, s:s + 1].rearrange("p t o -> p (t o)"),
                            op=OP.mult)

                def mkpsb(ch):
                    psb = ps_nrm.tile([128, FC], f32, tag="nps")
                    nc.tensor.matmul(out=psb[:], lhsT=ones_rowb[:],
                                     rhs=rin[:, ch * FC:(ch + 1) * FC],
                                     start=True, stop=True)
                    return psb
                psbs = []
                for ch in range(NFC):
                    psb = mkpsb(ch)
                    psbs.append(psb)
                    pack(range(_BS, S), ch, psb)

                # ---------- gather z2 -> Bn: one-hot shift-accumulate ----
                # Bn[:,t] = sum_d mask_d[t] * Z2f[:, 4+t+d]; the clamped
                # index computation guarantees masked-out terms never pick
                # the padding or a neighboring row's data
                nc.vector.tensor_tensor(Bn[:], Z2f[:, 1:NLOC + 1],
                                        masks[0], op=OP.mult)
                for dd in range(1, 2 * _W + 2):
                    nc.vector.tensor_tensor(
                        tmpb, Z2f[:, 1 + dd:NLOC + 1 + dd],
                        masks[dd], op=OP.mult)
                    nc.vector.tensor_add(Bn[:], Bn[:], tmpb)
                dma(out=ag_in_d.ap(), in_=Bn[:])
                nc.gpsimd.collective_compute(
                    "AllGather", OP.bypass,
                    replica_groups=[list(range(_NCORES))],
                    ins=[ag_in_d.ap().opt()],
                    outs=[ag_out_d.ap().opt()])

                # pack An while the collectives run
                for ch in range(NFC):
                    pack(range(_BS), ch, psbs[ch])

            Ball = bigbuf.tile([128, NGLOB], bf16, tag="ball")
            for jj in range(_NCORES):
                dma(out=Ball[:, jj * NLOC:(jj + 1) * NLOC],
                    in_=ag_out_d.ap()[jj * 128:(jj + 1) * 128, :])

            # ---------- output partials ----------
            outp = consts.tile([128, 4], f32)
            nc.vector.memset(outp[:], 0.0)

            with (
                tc.tile_pool(name="ps_d", bufs=1, space="PSUM") as ps_d,
                tc.tile_pool(name="ps_s", bufs=2, space="PSUM") as ps_s,
                tc.tile_pool(name="ps_c", bufs=1, space="PSUM") as ps_c,
            ):
                # diag: sum_i <An_i, Bn_i>
                dg = bigbuf.tile([128, NLOC], bf16, tag="dg")
                nc.vector.tensor_mul(dg[:], An[:], Bn[:])
                ndc = (NLOC + 511) // 512
                psd = ps_d.tile([1, 512], f32, tag="psd")
                for ch in range(ndc):
                    nc.tensor.matmul(out=psd[:], lhsT=ones_col[:],
                                     rhs=dg[:, ch * 512:(ch + 1) * 512],
                                     start=(ch == 0), stop=(ch == ndc - 1))
                nc.vector.reduce_sum(outp[0:1, 2:3], psd[:],
                                     axis=mybir.AxisListType.X)

                # one-pass row+col logsumexp sweep ([128,1024] exp
                # blocks; row sums on DVE, col sums via ones-matmuls)
                CP = 2 * CC
                NCP = NGLOB // CP
                rows = bigbuf.tile([128, NRC * NCP], f32, tag="rows")
                csum = nrm.tile([1, NGLOB], f32, tag="csum")
                for cb in range(NCP):
                    csp0 = ps_c.tile([1, CC], f32, tag="csp0")
                    csp1 = ps_c.tile([1, CC], f32, tag="csp1")
                    # exp blocks accumulate across row-chunks on DVE (freed
                    # by accum_out) so only ONE ones-matmul pair per column
                    # chunk is needed instead of NRC chained pairs
                    acc = sgp.tile([128, CP], bf16, tag="acc")
                    for rc in range(NRC):
                        ps = ps_s.tile([128, CP], f32, tag="sps")
                        nc.tensor.matmul(
                            out=ps[:, 0:CC],
                            lhsT=An[:, rc * 128:(rc + 1) * 128],
                            rhs=Ball[:, cb * CP:cb * CP + CC],
                            start=True, stop=True)
                        nc.tensor.matmul(
                            out=ps[:, CC:CP],
                            lhsT=An[:, rc * 128:(rc + 1) * 128],
                            rhs=Ball[:, cb * CP + CC:(cb + 1) * CP],
                            start=True, stop=True)
                        eb = sgp.tile([128, CP], bf16, tag="eb")
                        nc.scalar.activation(
                            eb[:], ps[:], AF.Exp, scale=1.0 / _TEMP,
                            accum_out=rows[:, rc * NCP + cb:
                                           rc * NCP + cb + 1])
                        if rc == 0:
                            nc.vector.tensor_copy(acc[:], eb[:])
                        else:
                            nc.vector.tensor_add(acc[:], acc[:], eb[:])
                    nc.tensor.matmul(
                        out=csp0[:], lhsT=ones_col[:], rhs=acc[:, 0:CC],
                        start=True, stop=True)
                    nc.tensor.matmul(
                        out=csp1[:], lhsT=ones_col[:], rhs=acc[:, CC:CP],
                        start=True, stop=True)
                    nc.vector.tensor_copy(csum[:, cb * CP:cb * CP + CC],
                                          csp0[:])
                    nc.vector.tensor_copy(
                        csum[:, cb * CP + CC:(cb + 1) * CP], csp1[:])

                tot = small.tile([128, NRC], f32, tag="tot")
                for rc in range(NRC):
                    nc.vector.reduce_sum(tot[:, rc:rc + 1],
                                         rows[:, rc * NCP:(rc + 1) * NCP],
                                         axis=mybir.AxisListType.X)
                lse = small.tile([128, NRC], f32, tag="lse")
                nc.scalar.activation(lse[:], tot[:], AF.Ln)
                nc.vector.reduce_sum(outp[:, 0:1], lse[:],
                                     axis=mybir.AxisListType.X)

            dma(out=out_d.ap(), in_=outp[:])
            dma(out=cs_d.ap(), in_=csum[:])
            if dbg:
                anb32 = bigbuf.tile([128, NLOC], f32, tag="anb32")
                for i, src in enumerate((An, Bn)):
                    nc.vector.tensor_copy(anb32[:], src[:])
                    dma(out=anb_d.ap()[:, i * NLOC:(i + 1) * NLOC],
                        in_=anb32[:])
                nc.vector.tensor_copy(anb32[:], Z2f[:, 4:NLOC + 4])
                dma(out=anb_d.ap()[:, 2 * NLOC:3 * NLOC], in_=anb32[:])
                nc.vector.tensor_copy(anb32[:], H1t[:, 0:NLOC])
                dma(out=anb_d.ap()[:, 3 * NLOC:4 * NLOC], in_=anb32[:])

    nc.compile()
    return nc


def _host_prep(x1, x2, u, wih0, whh0, bih0, bhh0, wih1, whh1, bih1, bhh1, T):
    """Build per-core input maps (all host work is pure data layout)."""
    import ml_dtypes
    bf16 = ml_dtypes.bfloat16

    # g-gate (kernel position _GTANH) weights/bias are pre-scaled by 2 so
    # a single sigmoid gives u = sigmoid(2g), tanh(g) = 2u - 1
    def gate_stack_T(w):
        # w: [2, 256, K] -> [K, 512]; cols = dir*256 + gperm_gate*64 + j
        K = w.shape[2]
        out = np.empty((K, 512), np.float32)
        for d in range(2):
            for gi, g in enumerate(_GPERM):
                sc = 2.0 if gi == _GTANH else 1.0
                out[:, d * 256 + gi * 64:d * 256 + (gi + 1) * 64] = \
                    sc * w[d, g * 64:(g + 1) * 64, :].T
        return out.astype(bf16)

    def blockdiag(w):
        # w: [2, 256, H] -> [128, 512]; per new-gate [128,128] block-diag
        out = np.zeros((128, 512), np.float32)
        for gi, g in enumerate(_GPERM):
            sc = 2.0 if gi == _GTANH else 1.0
            out[0:_H, gi * 128:gi * 128 + 64] = \
                sc * w[0, g * 64:(g + 1) * 64, :].T
            out[_H:128, gi * 128 + 64:(gi + 1) * 128] = \
                sc * w[1, g * 64:(g + 1) * 64, :].T
        return out.astype(bf16)

    def biases(bi, bh):
        b = bi + bh  # [2, 256]
        out = np.empty((128, 4), np.float32)
        for gi, g in enumerate(_GPERM):
            sc = 2.0 if gi == _GTANH else 1.0
            out[0:_H, gi] = sc * b[0, g * 64:(g + 1) * 64]
            out[_H:128, gi] = sc * b[1, g * 64:(g + 1) * 64]
        return out

    twt = np.tile(np.arange(T, dtype=np.float32), _BS)[None, :]

    shared = {
        "wi0T": np.ascontiguousarray(gate_stack_T(wih0)),
        "wi1T": np.ascontiguousarray(gate_stack_T(wih1)),
        "wh0bd": blockdiag(whh0),
        "wh1bd": blockdiag(whh1),
        "bias0": biases(bih0, bhh0),
        "bias1": biases(bih1, bhh1),
        "tw2": np.ascontiguousarray(twt),
    }
    in_maps = []
    for k in range(_NCORES):
        rows = [x1[2 * k, :T], x1[2 * k + 1, :T], x2[2 * k, :T],
                x2[2 * k + 1, :T]]
        arr = np.stack(rows, axis=2)            # [T, D, S]
        xc = np.ascontiguousarray(
            arr.transpose(1, 0, 2).reshape(_D, T * _S)).astype(bf16)
        xr = np.ascontiguousarray(
            arr[::-1].transpose(1, 0, 2).reshape(_D, T * _S)).astype(bf16)
        m = dict(shared)
        m["x_cat"] = xc
        m["x_rev"] = xr
        m["u2"] = np.ascontiguousarray(
            u[2 * k:2 * k + 2, :T].reshape(1, -1))
        in_maps.append(m)
    return in_maps


def _run(inputs, T=_T, trace=False, dbg=False):
    from concourse import bass_utils
    key = (T, dbg)
    if key not in _cache:
        _cache[key] = _build(T, dbg)
    nc = _cache[key]
    in_maps = _host_prep(T=T, **inputs)
    res = bass_utils.run_bass_kernel_spmd(
        nc, in_maps, core_ids=list(range(_NCORES)), trace=trace)
    N = _NCORES * _BS * T
    R = sum(float(r["outp"][:, 0].sum()) for r in res.results)
    Draw = sum(float(r["outp"][0, 2]) for r in res.results)
    colsum = np.zeros(N, np.float64)
    for r in res.results:
        colsum += np.asarray(r["colsum"][0], np.float64)
    C = float(np.log(colsum).sum())
    Dg = Draw / _TEMP
    loss = -((Dg - R) / N + (Dg - C) / N)
    return np.float32(loss), res


def kernel(**inputs):
    loss, _ = _run(inputs)
    return np.asarray(loss, dtype=np.float32)

